# revision 8
# baseline (speedup 1.0000x reference)
"""Sparse masked multi-head attention (GNN message passing) on 8 Trainium2 cores.

Strategy: shard query rows across cores (degree-balanced snake order).
Per core: project K/V for all nodes (node-major, bf16) into an HBM table,
gather per-edge K||V rows with indirect DMA, compute per-edge scores on DVE
(bf16 mul + segmented reduce), segment softmax (no max-shift needed: |s/8|<~3),
weight V by attention (DVE), segment-sum via identity-matmul PSUM accumulation,
then the output projection in fp32r on the PE.  Bias handling: bq added on
device; bk folded into scores via q.bk term; bv and bo folded into a constant
added on the host (out = attn_avg(v')+bv contributes (Wo@bv+bo) to y).
"""

import os
import numpy as np
import ml_dtypes

N = 8192
CH = 256
HEADS = 4
HD = 64
NCORES = 8
RPC = N // NCORES          # rows per core
CHUNKS = RPC // 128        # 8 chunks of 128 rows

_bf = ml_dtypes.bfloat16


def _prep(x, edge_index, Wq, bq, Wk, bk, Wv, bv, Wo, bo):
    """Host-side preprocessing: dedupe edges, degree-sort rows, build per-core
    gather indices / pad masks and dtype-converted weights."""
    x = np.asarray(x, dtype=np.float32)
    ei = np.asarray(edge_index)
    rows_raw = ei[0].astype(np.int64)
    cols_raw = ei[1].astype(np.int64)
    keys = np.unique(rows_raw * N + cols_raw)
    rows = (keys // N).astype(np.int32)
    cols = (keys % N).astype(np.int32)
    deg = np.bincount(rows, minlength=N).astype(np.int64)
    starts = np.zeros(N + 1, dtype=np.int64)
    np.cumsum(deg, out=starts[1:])

    order = np.argsort(-deg, kind="stable")
    perm = np.empty((NCORES, RPC), dtype=np.int64)
    for c in range(NCORES):
        perm[c] = order[c::NCORES]

    D = []
    for j in range(CHUNKS):
        mx = int(deg[perm[:, j * 128:(j + 1) * 128]].max())
        D.append(max(4, (mx + 3) // 4 * 4))
    assert max(D) <= 128, f"degree too large: {D}"
    SD = sum(D)

    idx = np.zeros((NCORES, 128, SD), dtype=np.int32)
    pmask = np.zeros((NCORES, 128, 4 * SD), dtype=_bf)
    for c in range(NCORES):
        off = 0
        for j in range(CHUNKS):
            dj = D[j]
            rws = perm[c, j * 128:(j + 1) * 128]
            for p, r in enumerate(rws):
                d = int(deg[r])
                idx[c, p, off:off + d] = cols[starts[r]:starts[r] + d]
                pm = np.zeros((dj, HEADS), dtype=np.float32)
                pm[:d, :] = 1.0
                pmask[c, p, 4 * off:4 * (off + dj)] = pm.reshape(-1).astype(_bf)
            off += dj

    # V channel swizzle: column ch' = d*HEADS + h of the table holds original
    # channel ch = h*HD + d, i.e. sw[ch'] = (ch' % HEADS)*HD + ch'//HEADS
    chp = np.arange(CH)
    sw = (chp % HEADS) * HD + (chp // HEADS)

    Wq = np.asarray(Wq, np.float32); Wk = np.asarray(Wk, np.float32)
    Wv = np.asarray(Wv, np.float32); Wo = np.asarray(Wo, np.float32)
    bq = np.asarray(bq, np.float32); bk = np.asarray(bk, np.float32)
    bv = np.asarray(bv, np.float32); bo = np.asarray(bo, np.float32)

    shared = {
        "xT": np.ascontiguousarray(x.T.astype(_bf)),                 # [256, 8192]
        "WkT": np.ascontiguousarray(Wk.T.astype(_bf)),                           # [in, out]
        "WvTs": np.ascontiguousarray(Wv.T[:, sw].astype(_bf)),                   # [in, out'] swizzled
        "WqT": np.ascontiguousarray(Wq.T.astype(_bf)),
        "WoTs": np.ascontiguousarray(Wo.T[sw, :].astype(_bf)),                   # [(d,h), out]
        "bk_rep": np.broadcast_to(bk.astype(_bf), (128, CH)).copy(),
        "bq_rep": np.broadcast_to(bq.astype(_bf), (128, CH)).copy(),
    }
    per_core = []
    for c in range(NCORES):
        per_core.append({
            "xTq": np.ascontiguousarray(x[perm[c]].T.astype(_bf)),   # [256, 1024]
            "idx": np.ascontiguousarray(idx[c]),
            "pmask": np.ascontiguousarray(pmask[c]),
        })
    y_const = (Wo @ bv + bo).astype(np.float32)                      # [256]
    return D, perm, shared, per_core, y_const


def _build(D):
    import concourse.bacc as bacc
    import concourse.bass as bass
    import concourse.mybir as mybir
    import concourse.tile as tile
    from concourse.masks import make_identity

    f32 = mybir.dt.float32
    f32r = mybir.dt.float32r
    bf16 = mybir.dt.bfloat16
    i32 = mybir.dt.int32
    SD = sum(D)

    nc = bacc.Bacc("TRN2", target_bir_lowering=False, debug=False,
                   num_devices=NCORES)

    xT_d = nc.dram_tensor("xT", [CH, N], bf16, kind="ExternalInput")
    xTq_d = nc.dram_tensor("xTq", [CH, RPC], bf16, kind="ExternalInput")
    WkT_d = nc.dram_tensor("WkT", [CH, CH], bf16, kind="ExternalInput")
    WvTs_d = nc.dram_tensor("WvTs", [CH, CH], bf16, kind="ExternalInput")
    WqT_d = nc.dram_tensor("WqT", [CH, CH], bf16, kind="ExternalInput")
    WoTs_d = nc.dram_tensor("WoTs", [CH, CH], bf16, kind="ExternalInput")
    bkrep_d = nc.dram_tensor("bk_rep", [128, CH], bf16, kind="ExternalInput")
    bqrep_d = nc.dram_tensor("bq_rep", [128, CH], bf16, kind="ExternalInput")
    idx_d = nc.dram_tensor("idx", [128, SD], i32, kind="ExternalInput")
    pm_d = nc.dram_tensor("pmask", [128, 4 * SD], bf16, kind="ExternalInput")
    y_d = nc.dram_tensor("y", [RPC, CH], f32, kind="ExternalOutput")
    kv_d = nc.dram_tensor("kv_table", [N, 2 * CH], bf16)   # internal HBM table

    with tile.TileContext(nc) as tc:
        with (
            tc.tile_pool(name="const", bufs=1) as cpool,
            tc.tile_pool(name="proj", bufs=2) as ppool,
            tc.tile_pool(name="psA", bufs=2, space="PSUM") as psA,
            tc.tile_pool(name="work", bufs=2) as wpool,
            tc.tile_pool(name="psB", bufs=1, space="PSUM") as psB,
            nc.allow_low_precision(reason="bf16 score/attn pipeline"),
        ):
            # ---- constants / inputs resident in SBUF ----
            xt = [cpool.tile([128, N], bf16, tag=f"xt{k}", name=f"xt{k}") for k in range(2)]
            for k in range(2):
                nc.sync.dma_start(out=xt[k][:], in_=xT_d[128 * k:128 * (k + 1), :])
            xq = [cpool.tile([128, RPC], bf16, tag=f"xq{k}", name=f"xq{k}") for k in range(2)]
            for k in range(2):
                nc.sync.dma_start(out=xq[k][:], in_=xTq_d[128 * k:128 * (k + 1), :])
            wk = [cpool.tile([128, CH], bf16, tag=f"wk{k}", name=f"wk{k}") for k in range(2)]
            wv = [cpool.tile([128, CH], bf16, tag=f"wv{k}", name=f"wv{k}") for k in range(2)]
            wq = [cpool.tile([128, CH], bf16, tag=f"wq{k}", name=f"wq{k}") for k in range(2)]
            wo = [cpool.tile([128, CH], bf16, tag=f"wo{k}", name=f"wo{k}") for k in range(2)]
            for k in range(2):
                sl = slice(128 * k, 128 * (k + 1))
                nc.sync.dma_start(out=wk[k][:], in_=WkT_d[sl, :])
                nc.sync.dma_start(out=wv[k][:], in_=WvTs_d[sl, :])
                nc.sync.dma_start(out=wq[k][:], in_=WqT_d[sl, :])
                nc.sync.dma_start(out=wo[k][:], in_=WoTs_d[sl, :])
            bkr = cpool.tile([128, CH], bf16, tag="bkr")
            bqr = cpool.tile([128, CH], bf16, tag="bqr")
            nc.sync.dma_start(out=bkr[:], in_=bkrep_d[:])
            nc.sync.dma_start(out=bqr[:], in_=bqrep_d[:])
            idxt = cpool.tile([128, SD], i32, tag="idx")
            nc.sync.dma_start(out=idxt[:], in_=idx_d[:])
            pmt = cpool.tile([128, 4 * SD], bf16, tag="pm")
            nc.sync.dma_start(out=pmt[:], in_=pm_d[:])
            ident_b = cpool.tile([128, 128], bf16, tag="idb")
            make_identity(nc, ident_b[:])
            ident_f = cpool.tile([128, 128], f32, tag="idf")
            make_identity(nc, ident_f[:])

            # ---- phase A: K/V table projection (node-major, no bias) ----
            for t in range(N // 128):
                pkv = psA.tile([128, 512], f32, tag="pkv")
                sl = slice(128 * t, 128 * (t + 1))
                for kc in range(2):
                    nc.tensor.matmul(out=pkv[:, 0:CH], lhsT=xt[kc][:, sl],
                                     rhs=wk[kc][:], start=(kc == 0), stop=(kc == 1))
                for kc in range(2):
                    nc.tensor.matmul(out=pkv[:, CH:2 * CH], lhsT=xt[kc][:, sl],
                                     rhs=wv[kc][:], start=(kc == 0), stop=(kc == 1))
                kvb = ppool.tile([128, 512], bf16, tag="kvb")
                if t % 2 == 0:
                    nc.scalar.activation(out=kvb[:], in_=pkv[:],
                                         func=mybir.ActivationFunctionType.Copy)
                else:
                    nc.vector.tensor_copy(out=kvb[:], in_=pkv[:])
                nc.sync.dma_start(out=kv_d[sl, :], in_=kvb[:])

            # ---- phase A': Q projection (node-major, +bq) ----
            qb = [cpool.tile([128, CH], bf16, tag=f"qb{j}", name=f"qb{j}") for j in range(CHUNKS)]
            for j in range(CHUNKS):
                pq = psA.tile([128, CH], f32, tag="pq")
                sl = slice(128 * j, 128 * (j + 1))
                for kc in range(2):
                    nc.tensor.matmul(out=pq[:], lhsT=xq[kc][:, sl], rhs=wq[kc][:],
                                     start=(kc == 0), stop=(kc == 1))
                nc.vector.tensor_tensor(out=qb[j][:], in0=pq[:], in1=bqr[:],
                                        op=mybir.AluOpType.add)

            # q.bk per (row, head): fold of K-bias into scores
            qbk = [cpool.tile([128, HEADS], f32, tag=f"qbk{j}", name=f"qbk{j}") for j in range(CHUNKS)]
            for j in range(CHUNKS):
                qkprod = wpool.tile([128, CH], bf16, tag="qkprod")
                nc.vector.tensor_tensor(out=qkprod[:], in0=qb[j][:], in1=bkr[:],
                                        op=mybir.AluOpType.mult)
                nc.vector.tensor_reduce(
                    out=qbk[j][:],
                    in_=qkprod[:].rearrange("p (h d) -> p h d", d=HD),
                    axis=mybir.AxisListType.X, op=mybir.AluOpType.add)

            # ---- phase B: per 128-row chunk sparse attention ----
            off = 0
            for j in range(CHUNKS):
                dj = D[j]
                kvg = wpool.tile([128, dj * 512], bf16, tag="kvg")
                kv3 = kvg[:].rearrange("p (j e) -> p j e", e=512)
                for jj in range(dj):
                    nc.gpsimd.indirect_dma_start(
                        out=kv3[:, jj, :], out_offset=None,
                        in_=kv_d[:],
                        in_offset=bass.IndirectOffsetOnAxis(
                            ap=idxt[:, off + jj:off + jj + 1], axis=0),
                    )
                # scores: PROD = KG * q (in-place on K half), S = seg-sum over d
                kview = kv3[:, :, 0:CH]
                nc.vector.tensor_tensor(
                    out=kview, in0=kview,
                    in1=qb[j][:][:, None, :].to_broadcast([128, dj, CH]),
                    op=mybir.AluOpType.mult)
                s_t = wpool.tile([128, dj * HEADS], bf16, tag="s")
                nc.vector.tensor_reduce(
                    out=s_t[:],
                    in_=kv3[:, :, 0:CH].rearrange("p j (h d) -> p j h d", d=HD),
                    axis=mybir.AxisListType.X, op=mybir.AluOpType.add)
                # S += q.bk (broadcast over j)
                s3 = s_t[:].rearrange("p (j h) -> p j h", h=HEADS)
                nc.vector.tensor_tensor(
                    out=s3, in0=s3,
                    in1=qbk[j][:][:, None, :].to_broadcast([128, dj, HEADS]),
                    op=mybir.AluOpType.add)
                # A = exp(S/8) * padmask
                a_t = wpool.tile([128, dj * HEADS], bf16, tag="a")
                nc.scalar.activation(out=a_t[:], in_=s_t[:],
                                     func=mybir.ActivationFunctionType.Exp,
                                     scale=0.125)
                nc.vector.tensor_tensor(out=a_t[:], in0=a_t[:],
                                        in1=pmt[:, 4 * off:4 * (off + dj)],
                                        op=mybir.AluOpType.mult)
                # Z = sum_j A  per head; RZ = 1/Z
                z_t = wpool.tile([128, HEADS], f32, tag="z")
                nc.vector.tensor_reduce(
                    out=z_t[:], in_=a_t[:].rearrange("p (j h) -> p h j", h=HEADS),
                    axis=mybir.AxisListType.X, op=mybir.AluOpType.add)
                rz_t = wpool.tile([128, HEADS], f32, tag="rz")
                nc.vector.reciprocal(out=rz_t[:], in_=z_t[:])
                # VGS = VG * A (in-place on V half; V channels are (d,h))
                vview = kv3[:, :, CH:2 * CH].rearrange("p j (d h) -> p j d h", h=HEADS)
                nc.vector.tensor_tensor(
                    out=vview, in0=vview,
                    in1=a_t[:].rearrange("p (j h) -> p j h", h=HEADS)[:, :, None, :]
                        .to_broadcast([128, dj, HD, HEADS]),
                    op=mybir.AluOpType.mult)
                # segment sum over j: identity-matmul PSUM accumulation
                pav = psB.tile([128, CH], f32, tag="pav", bufs=2)
                for jj in range(dj):
                    nc.tensor.matmul(out=pav[:], lhsT=ident_b[:],
                                     rhs=kv3[:, jj, CH:2 * CH],
                                     start=(jj == 0), stop=(jj == dj - 1))
                # normalize while evacuating PSUM
                oa = wpool.tile([128, CH], f32, tag="oa")
                nc.vector.tensor_tensor(
                    out=oa[:].rearrange("p (d h) -> p d h", h=HEADS),
                    in0=pav[:].rearrange("p (d h) -> p d h", h=HEADS),
                    in1=rz_t[:][:, None, :].to_broadcast([128, HD, HEADS]),
                    op=mybir.AluOpType.mult)
                # transpose out_attn -> [hd, r] for the output projection
                oaT = [wpool.tile([128, 128], bf16, tag=f"oaT{h}", name=f"oaT{h}") for h in range(2)]
                for h in range(2):
                    pt = psB.tile([128, 128], f32, tag="pt")
                    nc.tensor.transpose(out=pt[:], in_=oa[:][:, 128 * h:128 * (h + 1)],
                                        identity=ident_f[:])
                    nc.scalar.activation(out=oaT[h][:], in_=pt[:],
                                         func=mybir.ActivationFunctionType.Copy)
                # y = out_attn @ WoTs  (fp32r)
                py = psB.tile([128, CH], f32, tag="py")
                for h in range(2):
                    nc.tensor.matmul(out=py[:], lhsT=oaT[h][:], rhs=wo[h][:],
                                     start=(h == 0), stop=(h == 1))
                y_sb = wpool.tile([128, CH], f32, tag="ysb")
                nc.scalar.activation(out=y_sb[:], in_=py[:],
                                     func=mybir.ActivationFunctionType.Copy)
                nc.sync.dma_start(out=y_d[128 * j:128 * (j + 1), :], in_=y_sb[:])
                off += dj

    nc.compile()
    return nc


def kernel(**inputs):
    from concourse.bass_utils import run_bass_kernel_spmd

    D, perm, shared, per_core, y_const = _prep(**inputs)
    nc = _build(D)
    in_maps = []
    for c in range(NCORES):
        m = {"xT": shared["xT"], "WkT": shared["WkT"], "WvTs": shared["WvTs"],
             "WqT": shared["WqT"], "WoTs": shared["WoTs"],
             "bk_rep": shared["bk_rep"], "bq_rep": shared["bq_rep"],
             "xTq": per_core[c]["xTq"], "idx": per_core[c]["idx"],
             "pmask": per_core[c]["pmask"]}
        in_maps.append(m)

    trace = bool(int(os.environ.get("KERNEL_TRACE", "0")))
    if trace:
        try:
            import sys, types
            import antenv
            if not hasattr(antenv, "axon_hooks"):
                mod = types.ModuleType("antenv.axon_hooks")
                _state = {"hook": None}
                mod.set_axon_ntff_profile_hook = lambda h: _state.__setitem__("hook", h)
                mod.get_axon_ntff_profile_hook = lambda: _state["hook"]
                sys.modules["antenv.axon_hooks"] = mod
                antenv.axon_hooks = mod
                from trn_agent_boot.trn_boot import _ntff_profile_via_ctypes
                mod.set_axon_ntff_profile_hook(
                    _ntff_profile_via_ctypes('/opt/axon/libaxon_pjrt.so'))
        except Exception as e:
            print("trace shim failed:", e)
            trace = False

    res = run_bass_kernel_spmd(nc, in_maps, core_ids=list(range(NCORES)),
                               trace=trace)
    kernel.last_exec_time_ns = res.exec_time_ns
    kernel.last_trace = res.instructions_and_trace[1] if res.instructions_and_trace else None

    y = np.empty((N, CH), dtype=np.float32)
    for c in range(NCORES):
        y[perm[c]] = res.results[c]["y"]
    y += y_const[None, :]
    return y
